# revision 1
# baseline (speedup 1.0000x reference)
"""Trainium2 Bass kernel for nn_DDGCRN (gnn_message_passing).

DDGCRN: two 12-step GRU-style encoders over B=16, N=8600 nodes, HID=64,
with a global node-pooling term (GFS) inside each gate, plus conv heads.

Sharding: data-parallel over batch. B=16 / 8 cores = 2 batch elems per
core; the GFS node-pooling sum is per-batch-element, so no collectives.

Per-core layout: feature-major. Wide tensors are [D, W], W = 2*8600
(col = b*N + n). State tile X is [76, W]: rows 0:64 = hidden state,
rows 64:76 = x_t for all 12 steps (loaded once per encoder). Weights
are host-expanded to 12 variants [76, Dout] (x-weight at row 64+t).

GFS pooled term: for the staged inputs affw==1, aw==nw==1/N, so
C[d,n] = affw[n,d]*aw[n]*nw[n] is rank-1 with a CONSTANT v-row. The
pooled correction u[d]*v0*pooled[d] is then per-partition and is folded
into the activation BIAS of the second matmul pass (computed per step
by a tiny DVE op). Non-(rank1 & const-v) inputs fall back to streaming
the full C (and affb) from DRAM.

Engine balance per step (cost-model driven): PE does the 4 matmul
passes; Act does sigmoid+tanh+some relu drains; DVE does the remaining
relu drains + part of the z*S / GRU-combine elementwise; Pool (gpsimd)
takes the rest of the elementwise. TT ops read operands at different
partition offsets directly (no realign copies).
"""

import numpy as np
import ml_dtypes
from contextlib import ExitStack

import concourse.bass as bass
import concourse.bacc as bacc
import concourse.tile as tile
from concourse import mybir
from concourse.bass_utils import run_bass_kernel_spmd

F32 = mybir.dt.float32
BF16 = mybir.dt.bfloat16
AX = mybir.AxisListType
OP = mybir.AluOpType
AF = mybir.ActivationFunctionType

# Problem constants (hardcoded; kernel.py must be self-contained)
B, T, N_FULL, HID, IN = 16, 12, 8600, 64, 1
GIN = IN + HID
NCORES = 8
BLOC = B // NCORES  # 2
KK = 64 + T  # 76 lhsT rows: 64 state + 12 x slots


def _chunks(total, size):
    out = []
    off = 0
    while off < total:
        w = min(size, total - off)
        out.append((off, w))
        off += w
    return out


def _rank1_constv(C):
    """C [D, M]. If C == outer(u, ones*v0) (rank-1 with constant v row),
    return u_eff [D] with C == u_eff[:, None]; else None."""
    d0, m0 = np.unravel_index(np.argmax(np.abs(C)), C.shape)
    piv = C[d0, m0]
    if abs(piv) < 1e-30:
        return np.zeros(C.shape[0], np.float64)
    u = C[:, m0].astype(np.float64)
    v = C[d0, :].astype(np.float64) / piv
    if not np.allclose(np.outer(u, v), C, rtol=1e-5, atol=1e-7 * abs(piv)):
        return None
    if not np.allclose(v, v[0], rtol=1e-6, atol=1e-9):
        return None
    return u * v[0]


def _prep_host(inputs, n=N_FULL, t_steps=T):
    """Host-side parameter prep. Weight matrices become 12 variants
    [76, Dout] stacked to [12*76, Dout]: rows 0:64 = state weights,
    row 64+t = x weight for that step, other x rows zero."""
    f32 = np.float32
    H = {"flags": {}}
    shared = {}

    def var12(w, t):
        # w [65, Dout] -> [76, Dout]: rows 0:64 state, row 64+t = x weight
        dout = w.shape[1]
        out = np.zeros((KK, dout), f32)
        out[:64] = w[1:]
        out[64 + t] = w[0]
        return out

    for e in range(2):
        gaW = np.asarray(inputs["gate_alignW"][e], f32)   # [65, 128]
        gw = np.asarray(inputs["gate_w"][e], f32)         # [65, 128]
        gab = np.asarray(inputs["gate_alignb"][e], f32)   # [128]
        gb = np.asarray(inputs["gate_b"][e], f32)         # [128]
        uaW = np.asarray(inputs["upd_alignW"][e], f32)    # [65, 64]
        uw = np.asarray(inputs["upd_w"][e], f32)          # [65, 64]
        uab = np.asarray(inputs["upd_alignb"][e], f32)    # [64]
        ub = np.asarray(inputs["upd_b"][e], f32)          # [64]

        # one packed weight array per encoder: [76, 12*384]; per step t the
        # column block [t*384, (t+1)*384) = [gh(128) | ga(128) | uh(64) | ua(64)]
        wall = np.zeros((KK, t_steps * 384), f32)
        for t in range(t_steps):
            o = t * 384
            wall[:, o:o + 128] = var12(gw, t)
            wall[:, o + 128:o + 256] = var12(gaW, t)
            wall[:, o + 256:o + 320] = var12(uw, t)
            wall[:, o + 320:o + 384] = var12(uaW, t)
        shared[f"wall{e}"] = wall.astype(ml_dtypes.bfloat16)

        # pooled-term factor: C[d, n] = affw[n, d] * aw[n] * nw[n]
        for kind, aff, aw, nw, dout in (
            ("g", inputs["gate_affw"][e], inputs["gate_aw"][e], inputs["gate_nw"][e], 128),
            ("u", inputs["upd_affw"][e], inputs["upd_aw"][e], inputs["upd_nw"][e], 64),
        ):
            scale = (np.asarray(aw, f32)[:, 0] * np.asarray(nw, f32)[0])  # [n]
            C = np.asarray(aff, f32).T * scale[None, :]  # [dout, n]
            ue = _rank1_constv(C)
            if ue is None:
                H["flags"][f"cfull_{kind}{e}"] = True
                u = np.zeros(dout, f32)
                if kind == "g":
                    shared[f"cg{e}"] = np.ascontiguousarray(C.astype(f32))
                else:
                    shared[f"cu2_{e}"] = np.ascontiguousarray(
                        np.concatenate([C, C], axis=0).astype(f32))
            else:
                H["flags"][f"cfull_{kind}{e}"] = False
                u = ue.astype(f32)
            if kind == "g":
                ug = u
            else:
                uu = u

        # affb fallback (AB tensors) — zero for staged inputs
        abg = np.asarray(inputs["gate_affb"][e], f32).T  # [128, n]
        abu = np.asarray(inputs["upd_affb"][e], f32).T   # [64, n]
        H["flags"][f"ab_g{e}"] = bool(np.any(abg))
        H["flags"][f"ab_u{e}"] = bool(np.any(abu))
        if H["flags"][f"ab_g{e}"]:
            shared[f"abg{e}"] = np.ascontiguousarray(abg)
        if H["flags"][f"ab_u{e}"]:
            shared[f"abu2_{e}"] = np.ascontiguousarray(
                np.concatenate([abu, abu], axis=0))

        # bias/scale vector columns
        szr_sign = np.concatenate([np.ones(64, f32), -np.ones(64, f32)])
        H[f"gb{e}"] = gb
        H[f"bzr{e}"] = np.concatenate([gab[:64], -gab[64:]])
        H[f"ugs{e}"] = ug * szr_sign          # sign-folded gate pooled coeff
        H[f"ub2_{e}"] = np.concatenate([ub, ub])
        H[f"uab2_{e}"] = np.concatenate([uab, uab])
        H[f"uu2_{e}"] = np.concatenate([uu, uu])

    cw = np.asarray(inputs["conv_w"], f32)  # [3, 12, 64]
    cb = np.asarray(inputs["conv_b"], f32)  # [3, 12]
    # head order: [src1(12) | out1(12)] so src1 sits at partitions 0:12
    shared["cw01"] = np.concatenate([cw[1].T, cw[0].T], axis=1).astype(
        ml_dtypes.bfloat16)                                       # [64, 2T]
    shared["cw2"] = np.ascontiguousarray(cw[2].T).astype(ml_dtypes.bfloat16)

    # cvec columns
    ncol = 16
    cvec = np.zeros((128, ncol), F := np.float32)
    cvec[:64, 0] = 1.0
    cvec[64:, 0] = -1.0
    cols = {"szr": 0}
    ci = 1
    for e in range(2):
        for nm in ("gb", "bzr", "ugs", "ub2_", "uab2_", "uu2_"):
            key = f"{nm}{e}"
            arr = H[key]
            cvec[: len(arr), ci] = arr
            cols[key] = ci
            ci += 1
    th = cb.shape[1]
    cvec[: 2 * th, ci] = np.concatenate([cb[1], cb[0]])
    cols["cb01"] = ci
    ci += 1
    cvec[:th, ci] = cb[2]
    cols["cb2"] = ci
    shared["cvec"] = cvec
    H["cols"] = cols
    H["shared"] = shared
    return H


def _build(H, n=N_FULL, t_steps=T, ch=1024, mmc=512, ttc=1728):
    """Build the single-core Bass program (same for all cores)."""
    W = BLOC * n
    flags = H["flags"]
    cols = H["cols"]
    nc = bacc.Bacc("TRN2", target_bir_lowering=False, debug=False)

    dram = {}
    for name, arr in H["shared"].items():
        dram[name] = nc.declare_dram_parameter(
            name, list(arr.shape), mybir.dt.from_np(arr.dtype), isOutput=False)
    src32 = nc.declare_dram_parameter("src32", [t_steps, W], F32, isOutput=False)
    srcbf = nc.declare_dram_parameter("srcbf", [t_steps, W], BF16, isOutput=False)
    out_d = nc.declare_dram_parameter("out", [t_steps, W], BF16, isOutput=True)
    o1d = nc.dram_tensor("o1d", [t_steps, W], BF16)
    xsbf = nc.dram_tensor("xsbf", [t_steps, W], BF16)

    CH_B = _chunks(n, ch)       # chunks within one batch-half
    hch = min(ch, 1024)
    CH_W = _chunks(W, hch)      # chunks over full width (head passes)
    CH_T = _chunks(n, ttc)      # coarser chunks for TT elementwise
    nchb = len(CH_B)

    with tile.TileContext(nc) as tc, ExitStack() as ctx:
        # ---- persistent tiles ----
        pers = ctx.enter_context(tc.tile_pool(name="pers", bufs=1))

        def ptile(shape, dtype, nm):
            return pers.tile(shape, dtype, name=nm, tag=nm)

        X = ptile([KK, W], BF16, "X")      # state 0:64, x rows 64:76
        X2 = ptile([KK, W], BF16, "X2")    # z*S 0:64, x rows 64:76
        ZQ = ptile([128, W], BF16, "ZQ")   # z rows 0:64, q rows 64:128
        QA = ptile([64, W], BF16, "QA")    # q realigned to rows 0:64
        HC = ptile([128, n], BF16, "HC")   # tanh out, batch-stacked
        HC1 = ptile([64, n], BF16, "HC1")  # b1 tanh realigned to rows 0:64
        CVEC = ptile(list(H["shared"]["cvec"].shape), F32, "CVEC")
        PARTS = ptile([128, 2 * nchb], F32, "PARTS")
        PARTS2 = ptile([128, nchb], F32, "PARTS2")
        PG = ptile([128, 2], F32, "PG")
        P2 = ptile([128, 2], F32, "P2")
        BIASV = ptile([128, 8], F32, "BIASV")  # (t%2)*4 + {g_b0, g_b1, u, -}
        th = t_steps
        CW01 = ptile([64, 2 * th], BF16, "CW01")
        CW2 = ptile([64, th], BF16, "CW2")
        nc.sync.dma_start(CVEC[:, :], dram["cvec"][:, :])
        nc.sync.dma_start(CW01[:, :], dram["cw01"][:, :])
        nc.sync.dma_start(CW2[:, :], dram["cw2"][:, :])

        def bias(key):
            return CVEC[:, cols[key]:cols[key] + 1]

        # ---- pools ----
        ps = ctx.enter_context(tc.tile_pool(name="ps", bufs=4, space="PSUM"))
        sb = ctx.enter_context(tc.tile_pool(name="sb", bufs=3))
        small = ctx.enter_context(tc.tile_pool(name="small", bufs=2))
        fpool = ctx.enter_context(tc.tile_pool(name="fpool", bufs=2))
        wpool = ctx.enter_context(tc.tile_pool(name="wpool", bufs=3))

        wts = {}

        def fetch_w(e, t):
            if t >= t_steps:
                return
            wt = wpool.tile([KK, 384], BF16, tag="wt", bufs=3)
            nc.sync.dma_start(wt[:, :],
                              dram[f"wall{e}"][:, t * 384:(t + 1) * 384])
            wts[(e, t)] = wt

        def wslice(e, t, which):
            offs = {"gh": (0, 128), "ga": (128, 128),
                    "uh": (256, 64), "ua": (320, 64)}
            c0, cn = offs[which]
            return wts[(e, t)][:, c0:c0 + cn]

        def mm_into(p, lhsT, rhs_tile, rhs_rows, coff, cw_, p_rows=None):
            for m0, mw in _chunks(cw_, mmc):
                rhs_ap = rhs_tile[rhs_rows, coff + m0:coff + m0 + mw]
                o = p[:, m0:m0 + mw] if p_rows is None else \
                    p[p_rows, m0:m0 + mw]
                nc.tensor.matmul(o, lhsT, rhs_ap, start=True, stop=True)

        def gfs_gate(e, t, par):
            # t=0: state is zero -> contract only the x rows (64:KK)
            rk = slice(64, KK) if t == 0 else slice(0, KK)
            wr = (slice(64, KK), slice(None)) if t == 0 else \
                 (slice(0, KK), slice(None))
            # pass 1: h = relu(X@w+b); only per-partition row sums kept
            wgh = wslice(e, t, "gh")[wr[0], :]
            wga = wslice(e, t, "ga")[wr[0], :]
            for b in range(BLOC):
                boff = b * n
                for ci, (c0, cw_) in enumerate(CH_B):
                    p = ps.tile([128, ch], F32, tag="ps")
                    mm_into(p, wgh, X, rk, boff + c0, cw_)
                    acc = PARTS[:, b * nchb + ci:b * nchb + ci + 1]
                    if ci < 3:
                        nc.scalar.activation(p[:, :cw_], p[:, :cw_], AF.Relu,
                                             bias=bias(f"gb{e}"),
                                             accum_out=acc)
                    else:
                        nc.vector.tensor_scalar(p[:, :cw_], p[:, :cw_],
                                                bias(f"gb{e}"), 0.0,
                                                op0=OP.add, op1=OP.max,
                                                accum_out=acc)
                # pooled_b + bias2_b immediately, so sigmoid_b starts ASAP.
                # b0 on DVE, b1 on Pool: keeps each batch's chain on the
                # queue that frees up first.
                red = nc.vector
                red.tensor_reduce(
                    PG[:, b:b + 1], PARTS[:, b * nchb:(b + 1) * nchb],
                    axis=AX.X, op=OP.add)
                if not flags[f"cfull_g{e}"]:
                    # bias2_b = bzr + ugs * pooled_b
                    red.scalar_tensor_tensor(
                        BIASV[:, par * 4 + b:par * 4 + b + 1],
                        PG[:, b:b + 1], bias(f"ugs{e}"), bias(f"bzr{e}"),
                        op0=OP.mult, op1=OP.add)
            # pass 2: res -> sigmoid -> ZQ (z rows 0:64, q rows 64:128)
            for b in range(BLOC):
                boff = b * n
                for c0, cw_ in CH_B:
                    p = ps.tile([128, ch], F32, tag="ps")
                    mm_into(p, wga, X, rk, boff + c0, cw_)
                    if flags[f"cfull_g{e}"]:
                        cgc = fpool.tile([128, ch], F32, tag="cgc")
                        nc.sync.dma_start(cgc[:, :cw_],
                                          dram[f"cg{e}"][:, c0:c0 + cw_])
                        nc.vector.scalar_tensor_tensor(
                            p[:, :cw_], cgc[:, :cw_], PG[:, b:b + 1],
                            p[:, :cw_], op0=OP.mult, op1=OP.add)
                        bv = bias(f"bzr{e}")
                    else:
                        bv = BIASV[:, par * 4 + b:par * 4 + b + 1]
                    if flags[f"ab_g{e}"]:
                        abc = fpool.tile([128, ch], F32, tag="abc")
                        nc.sync.dma_start(abc[:, :cw_],
                                          dram[f"abg{e}"][:, c0:c0 + cw_])
                        nc.vector.tensor_add(p[:, :cw_], p[:, :cw_],
                                             abc[:, :cw_])
                    nc.scalar.activation(ZQ[:, boff + c0:boff + c0 + cw_],
                                         p[:, :cw_], AF.Sigmoid,
                                         bias=bv, scale=bias("szr"))
                # realign q for this batch to partition rows 0:64 (idle SP)
                nc.sync.dma_start(QA[:, boff:boff + n],
                                  ZQ[64:128, boff:boff + n])
                if t > 0:
                    # z*S -> X2 rows 0:64 (Pool: DVE is the max-loaded engine)
                    zeng = nc.gpsimd
                    for c0, cw_ in CH_T:
                        csl = slice(boff + c0, boff + c0 + cw_)
                        zeng.tensor_mul(X2[0:64, csl], ZQ[0:64, csl],
                                        X[0:64, csl])

        def gfs_upd(e, t, par):
            rk = slice(64, KK) if t == 0 else slice(0, KK)
            wuh = wslice(e, t, "uh")[rk, :]
            wua = wslice(e, t, "ua")[rk, :]
            # pass 1: h2 = relu(X2@uw+ub) batch-stacked; keep row sums
            for ci, (c0, cw_) in enumerate(CH_B):
                p = ps.tile([128, ch], F32, tag="ps")
                for b in range(BLOC):
                    mm_into(p, wuh, X2, rk, b * n + c0, cw_,
                            p_rows=slice(b * 64, b * 64 + 64))
                acc = PARTS2[:, ci:ci + 1]
                if ci < 2:
                    nc.scalar.activation(p[:, :cw_], p[:, :cw_], AF.Relu,
                                         bias=bias(f"ub2_{e}"),
                                         accum_out=acc)
                else:
                    nc.vector.tensor_scalar(p[:, :cw_], p[:, :cw_],
                                            bias(f"ub2_{e}"), 0.0,
                                            op0=OP.add, op1=OP.max,
                                            accum_out=acc)
            nc.vector.tensor_reduce(P2[:, 0:1], PARTS2[:, :],
                                    axis=AX.X, op=OP.add)
            if not flags[f"cfull_u{e}"]:
                # bias3 = uab2 + uu2 * pooled2  (both batch halves at once)
                nc.vector.scalar_tensor_tensor(
                    BIASV[:, par * 4 + 2:par * 4 + 3], P2[:, 0:1],
                    bias(f"uu2_{e}"), bias(f"uab2_{e}"),
                    op0=OP.mult, op1=OP.add)
            # pass 2: res2 -> tanh -> HC; realign b1 rows to HC1
            for ci, (c0, cw_) in enumerate(CH_B):
                p = ps.tile([128, ch], F32, tag="ps")
                for b in range(BLOC):
                    mm_into(p, wua, X2, rk, b * n + c0, cw_,
                            p_rows=slice(b * 64, b * 64 + 64))
                if flags[f"cfull_u{e}"]:
                    cuc = fpool.tile([128, ch], F32, tag="cgc")
                    nc.sync.dma_start(cuc[:, :cw_],
                                      dram[f"cu2_{e}"][:, c0:c0 + cw_])
                    nc.vector.scalar_tensor_tensor(
                        p[:, :cw_], cuc[:, :cw_], P2[:, 0:1], p[:, :cw_],
                        op0=OP.mult, op1=OP.add)
                    bv = bias(f"uab2_{e}")
                else:
                    bv = BIASV[:, par * 4 + 2:par * 4 + 3]
                if flags[f"ab_u{e}"]:
                    abc = fpool.tile([128, ch], F32, tag="abc")
                    nc.sync.dma_start(abc[:, :cw_],
                                      dram[f"abu2_{e}"][:, c0:c0 + cw_])
                    nc.vector.tensor_add(p[:, :cw_], p[:, :cw_], abc[:, :cw_])
                nc.scalar.activation(HC[:, c0:c0 + cw_], p[:, :cw_], AF.Tanh,
                                     bias=bv)
                nc.sync.dma_start(HC1[:, c0:c0 + cw_],
                                  HC[64:128, c0:c0 + cw_])
            # combine: X = X + q*(hc - X); all operands at rows 0:64.
            # b0 first so next step's gate pass unblocks column-early.
            for b in range(BLOC):
                boff = b * n
                for c0, cw_ in CH_T:
                    hsl = slice(c0, c0 + cw_)
                    xsl = slice(boff + c0, boff + c0 + cw_)
                    hcb = HC[0:64, hsl] if b == 0 else HC1[0:64, hsl]
                    if t == 0:
                        # X == 0: X_new = q * hc (single op; no memset needed)
                        zeng = nc.vector if b == 0 else nc.gpsimd
                        zeng.tensor_mul(X[0:64, xsl], QA[0:64, xsl], hcb)
                        continue
                    # X2 state rows are dead after the upd matmuls: scratch
                    d = X2[0:64, xsl]
                    ci = c0 // ttc
                    if b == 0:
                        ceng = nc.vector if ci != 2 else nc.gpsimd
                    else:
                        ceng = nc.gpsimd if ci != 2 else nc.vector
                    ceng.tensor_sub(d, hcb, X[0:64, xsl])
                    ceng.tensor_mul(d, QA[0:64, xsl], d)
                    ceng.tensor_add(X[0:64, xsl], X[0:64, xsl], d)

        def encoder(e, xbf_src):
            nc.sync.dma_start(X[64:KK, 0:n], xbf_src[:, 0:n])
            nc.sync.dma_start(X[64:KK, n:W], xbf_src[:, n:W])
            # X2 x-rows copied on DVE (4x bf16) in parallel with the SP DMA
            nc.vector.tensor_copy(X2[64:KK, :], X[64:KK, :])
            fetch_w(e, 0)
            fetch_w(e, 1)
            for t in range(t_steps):
                par = t % 2
                fetch_w(e, t + 2)
                gfs_gate(e, t, par)
                gfs_upd(e, t, par)

        # ================= encoder 1 =================
        encoder(0, srcbf)

        # heads 1+2 and encoder-2 input build
        for c0, cw_ in CH_W:
            p = ps.tile([2 * th, hch], F32, tag="ps")
            mm_into(p, CW01[:, :], X, slice(0, 64), c0, cw_)
            oc = sb.tile([2 * th, hch], BF16, tag="hc", bufs=2)
            nc.scalar.activation(oc[:, :cw_], p[: 2 * th, :cw_], AF.Identity,
                                 bias=CVEC[0: 2 * th, cols["cb01"]:cols["cb01"] + 1])
            # rows 0:12 = src1 (head 1), rows 12:24 = out1 (head 0)
            nc.sync.dma_start(o1d[:, c0:c0 + cw_], oc[th: 2 * th, :cw_])
            sc = small.tile([th, hch], BF16, tag="srcc")
            nc.sync.dma_start(sc[:, :cw_], srcbf[:, c0:c0 + cw_])
            xbb = sb.tile([th, hch], BF16, tag="xbb", bufs=2)
            nc.vector.tensor_sub(xbb[:, :cw_], sc[:, :cw_], oc[0:th, :cw_])
            nc.sync.dma_start(xsbf[:, c0:c0 + cw_], xbb[:, :cw_])

        # ================= encoder 2 =================
        encoder(1, xsbf)

        # head 3 + final sum
        for c0, cw_ in CH_W:
            p = ps.tile([th, hch], F32, tag="ps")
            mm_into(p, CW2[:, :], X, slice(0, 64), c0, cw_)
            o2 = sb.tile([th, hch], BF16, tag="hc2", bufs=2)
            nc.scalar.activation(o2[:, :cw_], p[:th, :cw_], AF.Identity,
                                 bias=CVEC[0:th, cols["cb2"]:cols["cb2"] + 1])
            o1c = sb.tile([th, hch], BF16, tag="d1", bufs=2)
            nc.sync.dma_start(o1c[:, :cw_], o1d[:, c0:c0 + cw_])
            nc.vector.tensor_add(o2[:, :cw_], o2[:, :cw_], o1c[:, :cw_])
            nc.sync.dma_start(out_d[:, c0:c0 + cw_], o2[:, :cw_])

    nc.compile()
    return nc


def _make_in_maps(inputs, H, n=N_FULL, t_steps=T):
    src = np.asarray(inputs["source"], np.float32)[..., 0]  # (B, T, n)
    in_maps = []
    for c in range(NCORES):
        m = dict(H["shared"])
        blk = src[BLOC * c: BLOC * (c + 1)]          # (BLOC, T, n)
        s = np.ascontiguousarray(
            blk.transpose(1, 0, 2).reshape(t_steps, BLOC * n))
        m["src32"] = s
        m["srcbf"] = s.astype(ml_dtypes.bfloat16)
        in_maps.append(m)
    return in_maps


def _assemble(results, n=N_FULL, t_steps=T):
    full = np.zeros((B, t_steps, n, 1), np.float32)
    for c in range(NCORES):
        o = np.asarray(results[c]["out"]).astype(np.float32)  # [T, BLOC*n]
        o = o.reshape(t_steps, BLOC, n).transpose(1, 0, 2)
        full[BLOC * c: BLOC * (c + 1), :, :, 0] = o
    return full


_PROG_CACHE = {}


def kernel(**inputs) -> np.ndarray:
    H = _prep_host(inputs)
    key = tuple(sorted(H["flags"].items()))
    if key not in _PROG_CACHE:
        _PROG_CACHE[key] = _build(H)
    nc = _PROG_CACHE[key]
    in_maps = _make_in_maps(inputs, H)
    res = run_bass_kernel_spmd(nc, in_maps, core_ids=list(range(NCORES)))
    return _assemble(res.results)

